# revision 10
# baseline (speedup 1.0000x reference)
"""ARAP loss kernel for Trainium2 (8 NeuronCores, SPMD).

loss[b] = mean_e |  ||x[b,dst]-x[b,src]||^2 - ||dx[b,dst]-dx[b,src]||^2  |

Strategy:
  * Directed edge list contains every non-self edge twice (both directions)
    and |q| is direction-symmetric, self loops contribute 0 -> compute each
    undirected edge (src<dst) once, scale by 2/E_dir.
  * Shard undirected edges across the 8 cores; every core holds the full
    vertex table with all 8 batch samples packed per row, so one gather
    descriptor serves all batches.
  * Vertex table row (256B): [x0(8b) x1(8b) x2(8b) dx0(8b) dx1(8b) dx2(8b)]
    f32 + 16 f32 zero pad. dma_gather needs int16 indices and 256B-aligned
    elements -> 4 banks of 25001 rows (25000 vertices + 1 zero row used for
    padding edges; padding gathers zeros for both endpoints -> q == 0).
  * Per-edge math: q = sum_c (A_c^2 - B_c^2) with A = xs-xd, B = dxs-dxd
    == sum_c (A-B)(A+B); sum |q| via tensor_reduce(apply_absolute_value).
  * Partition reduction via PE matmul with a ones vector, then AllReduce.
"""

import math

import numpy as np

import concourse.bass as bass
import concourse.bacc as bacc
import concourse.mybir as mybir
import concourse.tile as tile
from concourse.bass_utils import run_bass_kernel_spmd

import os
USE_CC = os.environ.get("ARAP_NO_CC", "") != "1"

B = 8
NCORES = 8
BANK = 25000
NBANK = 4
BROWS = BANK + 1          # rows per bank (last row = zeros)
ROWE = 64                 # f32 per table row (256B)
USED = 48                 # f32 actually used per row
MAXCALL = 2048            # max indices per dma_gather call

_CACHE = {}


def _plan(nv, e_dir):
    """Data-independent capacities for the 10 (sb<=db) groups per core."""
    e_und = e_dir // 2
    capd = _ru(e_und // 16 // NCORES + 768, 128)   # sb==db groups
    capo = _ru(e_und // 8 // NCORES + 768, 128)    # sb<db groups
    groups = [(sb, db) for sb in range(NBANK) for db in range(sb, NBANK)]
    caps = [capd if sb == db else capo for (sb, db) in groups]
    return groups, caps


def _ru(x, m):
    return (x + m - 1) // m * m


def _wrap16(a):
    """Logical index array (EC,) int16 -> [128, EC/16] tile layout
    (i -> [i%16, i//16], replicated across the 8 Q7 core blocks)."""
    blk = a.reshape(-1, 16).T  # (16, EC/16)
    return np.tile(blk, (8, 1)).copy()


def _build_nc(ec, caps, call_sizes, e_dir):
    nc = bacc.Bacc("TRN2", target_bir_lowering=False, debug=False,
                   num_devices=NCORES, num_swdge_queues=4)
    f32 = mybir.dt.float32
    xdx = nc.dram_tensor("xdx", [NBANK * BROWS, ROWE], f32, kind="ExternalInput")
    sidx = nc.dram_tensor("sidx", [128, ec // 16], mybir.dt.int16,
                          kind="ExternalInput")
    didx = nc.dram_tensor("didx", [128, ec // 16], mybir.dt.int16,
                          kind="ExternalInput")
    y = nc.dram_tensor("y", [B], f32, kind="ExternalOutput")
    cc_in = nc.dram_tensor("cc_in", [B], f32)
    cc_out = nc.dram_tensor("cc_out", [B], f32, addr_space="Shared")

    banks = [xdx[b * BROWS:(b + 1) * BROWS, :] for b in range(NBANK)]

    with tile.TileContext(nc) as tc:
        with (
            tc.tile_pool(name="gp", bufs=8) as gp,
            tc.tile_pool(name="cp", bufs=4) as cp,
            tc.tile_pool(name="const", bufs=1) as const,
            tc.tile_pool(name="psum", bufs=1, space="PSUM") as psum,
        ):
            sidx_t = const.tile([128, ec // 16], mybir.dt.int16)
            didx_t = const.tile([128, ec // 16], mybir.dt.int16)

            acc0 = const.tile([128, B], f32, tag="acc0")
            acc1 = const.tile([128, B], f32, tag="acc1")
            acc2 = const.tile([128, B], f32, tag="acc2")
            acc3 = const.tile([128, B], f32, tag="acc3")
            accs = [acc0, acc1, acc2, acc3]
            for a in accs:
                nc.vector.memset(a[:], 0.0)

            qn = 0
            for (p0, ch, sb, db) in call_sizes:
                k = ch // 128
                c0, c1 = p0 // 16, (p0 + ch) // 16
                gs = gp.tile([128, k * ROWE], f32, tag="gs")
                gd = gp.tile([128, k * ROWE], f32, tag="gd")
                nc.sync.dma_start(out=sidx_t[:, c0:c1], in_=sidx[:, c0:c1])
                nc.sync.dma_start(out=didx_t[:, c0:c1], in_=didx[:, c0:c1])
                nc.gpsimd.dma_gather(
                    out_ap=gs[:].rearrange("p (k c) -> p k c", c=ROWE),
                    in_ap=banks[sb], idxs_ap=sidx_t[:, c0:c1],
                    num_idxs=ch, num_idxs_reg=ch, elem_size=ROWE,
                    single_packet=False, queue_num=qn % 4)
                nc.gpsimd.dma_gather(
                    out_ap=gd[:].rearrange("p (k c) -> p k c", c=ROWE),
                    in_ap=banks[db], idxs_ap=didx_t[:, c0:c1],
                    num_idxs=ch, num_idxs_reg=ch, elem_size=ROWE,
                    single_packet=False, queue_num=(qn + 1) % 4)
                qn += 2

                d = cp.tile([128, k * USED], f32, tag="d")
                d3 = d[:].rearrange("p (k c) -> p k c", c=USED)
                g3s = gs[:].rearrange("p (k c) -> p k c", c=ROWE)
                g3d = gd[:].rearrange("p (k c) -> p k c", c=ROWE)
                nc.vector.tensor_tensor(out=d3, in0=g3s[:, :, 0:USED],
                                        in1=g3d[:, :, 0:USED],
                                        op=mybir.AluOpType.subtract)
                a_v = d3[:, :, 0:24]
                b_v = d3[:, :, 24:48]
                pt = cp.tile([128, k * 24], f32, tag="pt")
                qt = cp.tile([128, k * 24], f32, tag="qt")
                p3 = pt[:].rearrange("p (k c) -> p k c", c=24)
                q3 = qt[:].rearrange("p (k c) -> p k c", c=24)
                nc.vector.tensor_tensor(out=p3, in0=a_v, in1=b_v,
                                        op=mybir.AluOpType.subtract)
                nc.vector.tensor_tensor(out=q3, in0=a_v, in1=b_v,
                                        op=mybir.AluOpType.add)
                rt = cp.tile([128, k * 24], f32, tag="rt")
                nc.vector.tensor_tensor(out=rt[:], in0=pt[:], in1=qt[:],
                                        op=mybir.AluOpType.mult)
                # rt layout per edge: [c(3) x b(8)]; sum over c -> q8
                r4 = rt[:].rearrange("p (k c b) -> p k c b", c=3, b=B)
                q8 = cp.tile([128, k * B], f32, tag="q8")
                q83 = q8[:].rearrange("p (k b) -> p k b", b=B)
                nc.vector.tensor_tensor(out=q83, in0=r4[:, :, 0, :],
                                        in1=r4[:, :, 1, :],
                                        op=mybir.AluOpType.add)
                nc.vector.tensor_tensor(out=q83, in0=q83, in1=r4[:, :, 2, :],
                                        op=mybir.AluOpType.add)
                # sum_k |q8| -> [128, B]
                part = cp.tile([128, B], f32, tag="part")
                qv = q8[:].rearrange("p (k b) -> p b k", b=B)
                nc.vector.tensor_reduce(out=part[:], in_=qv,
                                        axis=mybir.AxisListType.X,
                                        op=mybir.AluOpType.add,
                                        apply_absolute_value=True)
                a_t = accs[(qn // 2) % 4]
                nc.vector.tensor_tensor(out=a_t[:], in0=a_t[:], in1=part[:],
                                        op=mybir.AluOpType.add)

            nc.vector.tensor_tensor(out=accs[0][:], in0=accs[0][:],
                                    in1=accs[1][:], op=mybir.AluOpType.add)
            nc.vector.tensor_tensor(out=accs[2][:], in0=accs[2][:],
                                    in1=accs[3][:], op=mybir.AluOpType.add)
            nc.vector.tensor_tensor(out=accs[0][:], in0=accs[0][:],
                                    in1=accs[2][:], op=mybir.AluOpType.add)
            ones = const.tile([128, 1], f32)
            nc.vector.memset(ones[:], 1.0)
            tot = psum.tile([B, 1], f32, space="PSUM")
            nc.tensor.matmul(out=tot[:], lhsT=accs[0][:], rhs=ones[:],
                             start=True, stop=True)
            res = const.tile([B, 1], f32)
            nc.scalar.mul(out=res[:], in_=tot[:], mul=2.0 / float(e_dir))
            if USE_CC:
                nc.sync.dma_start(out=cc_in[:], in_=res[:, 0])
                nc.gpsimd.collective_compute(
                    "AllReduce", mybir.AluOpType.add,
                    replica_groups=[list(range(NCORES))],
                    ins=[cc_in[:]], outs=[cc_out[:]])
                nc.sync.dma_start(out=y[:], in_=cc_out[:])
            else:
                nc.sync.dma_start(out=y[:], in_=res[:, 0])

    nc.compile()
    return nc


def kernel(dx, x, edge_src, edge_dst):
    dx = np.asarray(dx)
    x = np.asarray(x)
    edge_src = np.asarray(edge_src)
    edge_dst = np.asarray(edge_dst)
    nb, nv, _ = x.shape
    assert nb == B
    e_dir = int(edge_src.shape[0])

    # ---- vertex table ----
    tbl = np.zeros((NBANK * BROWS, ROWE), dtype=np.float32)
    feat = np.concatenate(
        [x.transpose(1, 2, 0), dx.transpose(1, 2, 0)], axis=1
    ).reshape(nv, USED)  # (NV, 48): [x0*8, x1*8, x2*8, dx0*8, dx1*8, dx2*8]
    for bk in range(NBANK):
        lo = bk * BANK
        hi = min(lo + BANK, nv)
        tbl[bk * BROWS: bk * BROWS + (hi - lo), :USED] = feat[lo:hi]

    # ---- undirected edges, bank groups ----
    mask = edge_src < edge_dst
    es = edge_src[mask].astype(np.int64)
    ed = edge_dst[mask].astype(np.int64)
    sb_all = es // BANK
    db_all = ed // BANK
    rs_all = (es % BANK).astype(np.int16)
    rd_all = (ed % BANK).astype(np.int16)

    groups = [(sb, db) for sb in range(NBANK) for db in range(sb, NBANK)]
    # tight per-group capacity = max per-core count, rounded to 128
    caps = []
    for (sb, db) in groups:
        n = int(((sb_all == sb) & (db_all == db)).sum())
        per = math.ceil(n / NCORES) if n else 0
        caps.append(max(128, _ru(per, 128)))

    # per (core, group) index arrays, padded with the bank zero row
    ec = sum(caps)
    src16 = np.full((NCORES, ec), BANK, dtype=np.int16)
    dst16 = np.full((NCORES, ec), BANK, dtype=np.int16)
    offs = np.cumsum([0] + caps)
    for gi, (sb, db) in enumerate(groups):
        sel = (sb_all == sb) & (db_all == db)
        rs, rd = rs_all[sel], rd_all[sel]
        n = len(rs)
        per = math.ceil(n / NCORES) if n else 0
        if per > caps[gi]:
            raise RuntimeError(
                f"group {(sb, db)} per-core count {per} > cap {caps[gi]}; "
                f"edge distribution too skewed for baked capacities")
        for c in range(NCORES):
            s = slice(c * per, min((c + 1) * per, n))
            cnt = max(0, s.stop - s.start)
            if cnt:
                src16[c, offs[gi]:offs[gi] + cnt] = rs[s]
                dst16[c, offs[gi]:offs[gi] + cnt] = rd[s]

    # call schedule: (start_pos, size, sb, db), sizes <= MAXCALL
    call_sizes = []
    for gi, (sb, db) in enumerate(groups):
        p0, rem = offs[gi], caps[gi]
        while rem > 0:
            ch = min(rem, MAXCALL)
            call_sizes.append((p0, ch, sb, db))
            p0 += ch
            rem -= ch

    key = (nv, e_dir, ec, tuple(caps))
    if key not in _CACHE:
        _CACHE[key] = _build_nc(ec, caps, call_sizes, e_dir)
    nc = _CACHE[key]

    in_maps = [
        {"xdx": tbl, "sidx": _wrap16(src16[c]), "didx": _wrap16(dst16[c])}
        for c in range(NCORES)
    ]
    res = run_bass_kernel_spmd(nc, in_maps, list(range(NCORES)), trace=False)
    if USE_CC:
        return res.results[0]["y"].astype(np.float32)
    return np.sum([res.results[c]["y"] for c in range(NCORES)], axis=0).astype(np.float32)


# revision 12
# speedup vs baseline: 1.0732x; 1.0732x over previous
"""ARAP loss kernel for Trainium2 (8 NeuronCores, SPMD).

loss[b] = mean_e |  ||x[b,dst]-x[b,src]||^2 - ||dx[b,dst]-dx[b,src]||^2  |

Strategy:
  * Directed edge list contains every non-self edge twice (both directions)
    and |q| is direction-symmetric, self loops contribute 0 -> compute each
    undirected edge (src<dst) once, scale by 2/E_dir.
  * Shard undirected edges across the 8 cores; every core holds the full
    vertex table with all 8 batch samples packed per row, so one gather
    descriptor serves all batches.
  * Vertex table row (256B): [x0(8b) x1(8b) x2(8b) dx0(8b) dx1(8b) dx2(8b)]
    f32 + 16 f32 zero pad. dma_gather needs int16 indices and 256B-aligned
    elements -> 4 banks of 25001 rows (25000 vertices + 1 zero row used for
    padding edges; padding gathers zeros for both endpoints -> q == 0).
  * Per-edge math: q = sum_c (A_c^2 - B_c^2) with A = xs-xd, B = dxs-dxd
    == sum_c (A-B)(A+B); sum |q| via tensor_reduce(apply_absolute_value).
  * Partition reduction via PE matmul with a ones vector, then AllReduce.
"""

import math

import numpy as np

import concourse.bass as bass
import concourse.bacc as bacc
import concourse.mybir as mybir
import concourse.tile as tile
from concourse.bass_utils import run_bass_kernel_spmd

import os
USE_CC = os.environ.get("ARAP_NO_CC", "") != "1"

B = 8
NCORES = 8
BANK = 25000
NBANK = 4
BROWS = BANK + 1          # rows per bank (last row = zeros)
ROWE = 64                 # f32 per table row (256B)
USED = 48                 # f32 actually used per row
MAXCALL = 1024            # max indices per dma_gather call

_CACHE = {}


def _plan(nv, e_dir):
    """Data-independent capacities for the 10 (sb<=db) groups per core."""
    e_und = e_dir // 2
    capd = _ru(e_und // 16 // NCORES + 768, 128)   # sb==db groups
    capo = _ru(e_und // 8 // NCORES + 768, 128)    # sb<db groups
    groups = [(sb, db) for sb in range(NBANK) for db in range(sb, NBANK)]
    caps = [capd if sb == db else capo for (sb, db) in groups]
    return groups, caps


def _ru(x, m):
    return (x + m - 1) // m * m


def _wrap16(a):
    """Logical index array (EC,) int16 -> [128, EC/16] tile layout
    (i -> [i%16, i//16], replicated across the 8 Q7 core blocks)."""
    blk = a.reshape(-1, 16).T  # (16, EC/16)
    return np.tile(blk, (8, 1)).copy()


def _build_nc(ec, caps, call_sizes, e_dir):
    nc = bacc.Bacc("TRN2", target_bir_lowering=False, debug=False,
                   num_devices=NCORES, num_swdge_queues=4)
    f32 = mybir.dt.float32
    xdx = nc.dram_tensor("xdx", [NBANK * BROWS, ROWE], f32, kind="ExternalInput")
    sidx = nc.dram_tensor("sidx", [128, ec // 16], mybir.dt.int16,
                          kind="ExternalInput")
    didx = nc.dram_tensor("didx", [128, ec // 16], mybir.dt.int16,
                          kind="ExternalInput")
    y = nc.dram_tensor("y", [B], f32, kind="ExternalOutput")
    cc_in = nc.dram_tensor("cc_in", [B], f32)
    cc_out = nc.dram_tensor("cc_out", [B], f32, addr_space="Shared")

    banks = [xdx[b * BROWS:(b + 1) * BROWS, :] for b in range(NBANK)]

    with tile.TileContext(nc) as tc:
        with (
            tc.tile_pool(name="gp", bufs=12) as gp,
            tc.tile_pool(name="cp", bufs=6) as cp,
            tc.tile_pool(name="const", bufs=1) as const,
            tc.tile_pool(name="psum", bufs=1, space="PSUM") as psum,
        ):
            sidx_t = const.tile([128, ec // 16], mybir.dt.int16)
            didx_t = const.tile([128, ec // 16], mybir.dt.int16)
            nc.sync.dma_start(out=sidx_t[:], in_=sidx[:])
            nc.sync.dma_start(out=didx_t[:], in_=didx[:])

            acc = const.tile([128, B], f32)
            nc.vector.memset(acc[:], 0.0)

            qn = 0
            for (p0, ch, sb, db) in call_sizes:
                k = ch // 128
                c0, c1 = p0 // 16, (p0 + ch) // 16
                gs = gp.tile([128, k * ROWE], f32, tag="gs")
                gd = gp.tile([128, k * ROWE], f32, tag="gd")
                nc.gpsimd.dma_gather(
                    out_ap=gs[:].rearrange("p (k c) -> p k c", c=ROWE),
                    in_ap=banks[sb], idxs_ap=sidx_t[:, c0:c1],
                    num_idxs=ch, num_idxs_reg=ch, elem_size=ROWE,
                    single_packet=False, queue_num=qn % 4)
                nc.gpsimd.dma_gather(
                    out_ap=gd[:].rearrange("p (k c) -> p k c", c=ROWE),
                    in_ap=banks[db], idxs_ap=didx_t[:, c0:c1],
                    num_idxs=ch, num_idxs_reg=ch, elem_size=ROWE,
                    single_packet=False, queue_num=(qn + 1) % 4)
                qn += 2

                d = cp.tile([128, k * ROWE], f32, tag="d")
                nc.vector.tensor_tensor(out=d[:], in0=gs[:], in1=gd[:],
                                        op=mybir.AluOpType.subtract)
                d3 = d[:].rearrange("p (k c) -> p k c", c=ROWE)
                a_v = d3[:, :, 0:24]
                b_v = d3[:, :, 24:48]
                pt = cp.tile([128, k * 24], f32, tag="pt")
                qt = cp.tile([128, k * 24], f32, tag="qt")
                p3 = pt[:].rearrange("p (k c) -> p k c", c=24)
                q3 = qt[:].rearrange("p (k c) -> p k c", c=24)
                nc.vector.tensor_tensor(out=p3, in0=a_v, in1=b_v,
                                        op=mybir.AluOpType.subtract)
                nc.vector.tensor_tensor(out=q3, in0=a_v, in1=b_v,
                                        op=mybir.AluOpType.add)
                rt = cp.tile([128, k * 24], f32, tag="rt")
                nc.vector.tensor_tensor(out=rt[:], in0=pt[:], in1=qt[:],
                                        op=mybir.AluOpType.mult)
                # rt layout per edge: [c(3) x b(8)]; sum over c -> q8
                r4 = rt[:].rearrange("p (k c b) -> p k c b", c=3, b=B)
                q8 = cp.tile([128, k * B], f32, tag="q8")
                q83 = q8[:].rearrange("p (k b) -> p k b", b=B)
                nc.vector.tensor_tensor(out=q83, in0=r4[:, :, 0, :],
                                        in1=r4[:, :, 1, :],
                                        op=mybir.AluOpType.add)
                nc.vector.tensor_tensor(out=q83, in0=q83, in1=r4[:, :, 2, :],
                                        op=mybir.AluOpType.add)
                # sum_k |q8| -> [128, B]
                part = cp.tile([128, B], f32, tag="part")
                qv = q8[:].rearrange("p (k b) -> p b k", b=B)
                nc.vector.tensor_reduce(out=part[:], in_=qv,
                                        axis=mybir.AxisListType.X,
                                        op=mybir.AluOpType.add,
                                        apply_absolute_value=True)
                nc.vector.tensor_tensor(out=acc[:], in0=acc[:], in1=part[:],
                                        op=mybir.AluOpType.add)

            ones = const.tile([128, 1], f32)
            nc.vector.memset(ones[:], 1.0)
            tot = psum.tile([B, 1], f32, space="PSUM")
            nc.tensor.matmul(out=tot[:], lhsT=acc[:], rhs=ones[:],
                             start=True, stop=True)
            res = const.tile([B, 1], f32)
            nc.scalar.mul(out=res[:], in_=tot[:], mul=2.0 / float(e_dir))
            if USE_CC:
                nc.sync.dma_start(out=cc_in[:], in_=res[:, 0])
                nc.gpsimd.collective_compute(
                    "AllReduce", mybir.AluOpType.add,
                    replica_groups=[list(range(NCORES))],
                    ins=[cc_in[:]], outs=[cc_out[:]])
                nc.sync.dma_start(out=y[:], in_=cc_out[:])
            else:
                nc.sync.dma_start(out=y[:], in_=res[:, 0])

    nc.compile()
    return nc


def kernel(dx, x, edge_src, edge_dst):
    dx = np.asarray(dx)
    x = np.asarray(x)
    edge_src = np.asarray(edge_src)
    edge_dst = np.asarray(edge_dst)
    nb, nv, _ = x.shape
    assert nb == B
    e_dir = int(edge_src.shape[0])

    # ---- vertex table ----
    tbl = np.zeros((NBANK * BROWS, ROWE), dtype=np.float32)
    feat = np.concatenate(
        [x.transpose(1, 2, 0), dx.transpose(1, 2, 0)], axis=1
    ).reshape(nv, USED)  # (NV, 48): [x0*8, x1*8, x2*8, dx0*8, dx1*8, dx2*8]
    for bk in range(NBANK):
        lo = bk * BANK
        hi = min(lo + BANK, nv)
        tbl[bk * BROWS: bk * BROWS + (hi - lo), :USED] = feat[lo:hi]

    # ---- undirected edges, bank groups ----
    mask = edge_src < edge_dst
    es = edge_src[mask].astype(np.int64)
    ed = edge_dst[mask].astype(np.int64)
    sb_all = es // BANK
    db_all = ed // BANK
    rs_all = (es % BANK).astype(np.int16)
    rd_all = (ed % BANK).astype(np.int16)

    groups = [(sb, db) for sb in range(NBANK) for db in range(sb, NBANK)]
    # tight per-group capacity = max per-core count, rounded to 128
    caps = []
    for (sb, db) in groups:
        n = int(((sb_all == sb) & (db_all == db)).sum())
        per = math.ceil(n / NCORES) if n else 0
        caps.append(max(128, _ru(per, 128)))

    # per (core, group) index arrays, padded with the bank zero row
    ec = sum(caps)
    src16 = np.full((NCORES, ec), BANK, dtype=np.int16)
    dst16 = np.full((NCORES, ec), BANK, dtype=np.int16)
    offs = np.cumsum([0] + caps)
    for gi, (sb, db) in enumerate(groups):
        sel = (sb_all == sb) & (db_all == db)
        rs, rd = rs_all[sel], rd_all[sel]
        n = len(rs)
        per = math.ceil(n / NCORES) if n else 0
        if per > caps[gi]:
            raise RuntimeError(
                f"group {(sb, db)} per-core count {per} > cap {caps[gi]}; "
                f"edge distribution too skewed for baked capacities")
        for c in range(NCORES):
            s = slice(c * per, min((c + 1) * per, n))
            cnt = max(0, s.stop - s.start)
            if cnt:
                src16[c, offs[gi]:offs[gi] + cnt] = rs[s]
                dst16[c, offs[gi]:offs[gi] + cnt] = rd[s]

    # call schedule: (start_pos, size, sb, db), sizes <= MAXCALL
    call_sizes = []
    for gi, (sb, db) in enumerate(groups):
        p0, rem = offs[gi], caps[gi]
        while rem > 0:
            ch = min(rem, MAXCALL)
            call_sizes.append((p0, ch, sb, db))
            p0 += ch
            rem -= ch

    key = (nv, e_dir, ec, tuple(caps))
    if key not in _CACHE:
        _CACHE[key] = _build_nc(ec, caps, call_sizes, e_dir)
    nc = _CACHE[key]

    in_maps = [
        {"xdx": tbl, "sidx": _wrap16(src16[c]), "didx": _wrap16(dst16[c])}
        for c in range(NCORES)
    ]
    res = run_bass_kernel_spmd(nc, in_maps, list(range(NCORES)), trace=False)
    if USE_CC:
        return res.results[0]["y"].astype(np.float32)
    return np.sum([res.results[c]["y"] for c in range(NCORES)], axis=0).astype(np.float32)
